# revision 2
# baseline (speedup 1.0000x reference)
"""GCN v3: v2 + merged slot layout per (super,k) with boundary-split S columns,
plus optional degree-balanced dest assignment.

Slot layout per (super, k): the 5 blocks' edges packed contiguously
(slots_bk = max-over-cores E_bk, equal on all cores), tail padded to a
128-chunk boundary with dummy idx.  A chunk overlapping 2 block segments
gets 2 S-columns (each with -1 outside its block's slot range), so every
matmul is a full-128-partition one targeting a single block's psum slice.

Matmul schedule (static, shared across cores): per (s, k) -> list of
(chunk_col, s_col, block) pieces.
"""

import numpy as np
import ml_dtypes

import concourse.bass as bass
import concourse.mybir as mybir
import concourse.tile as tile
from concourse import bacc
from concourse.bass_utils import run_bass_kernel_spmd

F = 128
CH = 128
N_CORES = 8
DBLK = 96
SRC_BLK = 32768
SUPER = 5
K_TT = 16
REBALANCE = True


def _assign_dests(r, n_nodes, npc, nblk):
    """dest -> (core, block, d). Returns perm[core*npc + local] = global dest.
    Degree-balanced: snake-deal dests (sorted by per-k degree profile) into
    8*nblk block bins."""
    if not REBALANCE:
        perm = np.arange(n_nodes, dtype=np.int64)
        return perm
    deg = np.bincount(r, minlength=n_nodes)
    order = np.argsort(-deg, kind="stable")
    nbins = N_CORES * nblk
    # snake deal into bins: bin sequence 0..nbins-1, nbins-1..0, ...
    perm = np.empty(n_nodes, dtype=np.int64)
    # bins[b] collects dests; bin b -> core b%8, block b//8
    bins = [[] for _ in range(nbins)]
    i = 0
    fwd = True
    while i < n_nodes:
        rng = range(nbins) if fwd else range(nbins - 1, -1, -1)
        for b in rng:
            if i >= n_nodes:
                break
            if len(bins[b]) < DBLK:
                bins[b].append(order[i])
                i += 1
        fwd = not fwd
    for b in range(nbins):
        core, blk = b % N_CORES, b // N_CORES
        for d, g in enumerate(bins[b]):
            loc = blk * DBLK + d
            if loc < npc:
                perm[core * npc + loc] = g
    return perm


def _prep(x, edge_index, weight, bias, n_nodes):
    r = np.asarray(edge_index[0], dtype=np.int64)
    c = np.asarray(edge_index[1], dtype=np.int64)
    deg = (np.bincount(r, minlength=n_nodes) + 1).astype(np.float64)
    dis = (1.0 / np.sqrt(deg)).astype(np.float32)

    x = np.asarray(x, dtype=np.float32)
    x_pre = (x * dis[:, None]).astype(ml_dtypes.bfloat16)
    xT_pre = np.ascontiguousarray(x_pre.T)

    npc = n_nodes // N_CORES
    nblk = (npc + DBLK - 1) // DBLK
    nk = (n_nodes + SRC_BLK - 1) // SRC_BLK
    nsup = (nblk + SUPER - 1) // SUPER

    perm = _assign_dests(r, n_nodes, npc, nblk)      # [8*npc] -> global dest
    inv = np.empty(n_nodes, np.int64)
    inv[perm] = np.arange(n_nodes)                   # global dest -> slot
    slot_of_r = inv[r]                               # edge -> output slot
    core = slot_of_r // npc
    rloc = slot_of_r - core * npc
    b_arr = rloc // DBLK
    d_arr = (rloc % DBLK).astype(np.float32)
    k_arr = c // SRC_BLK

    gid = (core * nblk + b_arr) * nk + k_arr
    counts = np.bincount(gid, minlength=N_CORES * nblk * nk).reshape(
        N_CORES, nblk, nk)
    slots_bk = counts.max(axis=0)                    # [nblk, nk] (no rounding)

    # layout per (s, k): blocks packed; chunk count per (s,k); schedule.
    seg_off = np.zeros((nblk, nk), np.int64)         # slot offset in its (s,k)
    sup_cols = np.zeros((nsup, nk), np.int64)
    schedule = {}                                    # (s,k) -> [(col, scol, b)]
    scol = 0
    scol_rows = []                                   # per s_col: (b,k,lo,hi) slot range in chunk
    for s in range(nsup):
        blocks = list(range(s * SUPER, min((s + 1) * SUPER, nblk)))
        for k in range(nk):
            o = 0
            for b in blocks:
                seg_off[b, k] = o
                o += int(slots_bk[b, k])
            cols = (o + CH - 1) // CH
            sup_cols[s, k] = cols
            sch = []
            for b in blocks:
                lo, hi = seg_off[b, k], seg_off[b, k] + int(slots_bk[b, k])
                col0, col1 = lo // CH, (hi - 1) // CH if hi > lo else lo // CH
                for col in range(col0, col1 + 1):
                    plo = max(lo - col * CH, 0)
                    phi = min(hi - col * CH, CH)
                    sch.append((col, scol, b, plo, phi))
                    scol_rows.append((s, k, b, col, plo, phi))
                    scol += 1
            schedule[(s, k)] = sch
    S_tot = scol
    S_pad = ((S_tot + K_TT - 1) // K_TT) * K_TT

    per_core = []
    for cid in range(N_CORES):
        sel = core == cid
        bs_, ks_, ds_, cs_ = b_arr[sel], k_arr[sel], d_arr[sel], c[sel]
        o = np.lexsort((ks_, bs_))
        bs_, ks_, ds_, cs_ = bs_[o], ks_[o], ds_[o], cs_[o]
        g = bs_ * nk + ks_
        cnt = np.bincount(g, minlength=nblk * nk)
        gstart = np.zeros(nblk * nk, np.int64)
        gstart[1:] = np.cumsum(cnt)[:-1]
        j = np.arange(len(g)) - gstart[g]            # rank within (b, k)

        gslot = seg_off[bs_, ks_] + j                # slot within (s, k)

        # idx arrays per (super, k)
        idx_arrs = {}
        sup_of_b = bs_ // SUPER
        for s in range(nsup):
            for k in range(nk):
                cols = int(sup_cols[s, k])
                arr = np.zeros(cols * CH, np.int16)
                m = (ks_ == k) & (sup_of_b == s)
                arr[gslot[m]] = (cs_[m] - k * SRC_BLK).astype(np.int16)
                wrapped = arr.reshape(-1, 16).T
                idx_arrs[f"idx_{s}_{k}"] = np.tile(wrapped, (8, 1)).copy()

        # rowlocT [128, S_pad]: for each s_col (piece), rowloc of its slots
        rlT = np.full((CH, S_pad), -1.0, np.float32)
        # edge -> its piece s_col: piece identified by (b, k, col)
        piece_id = {}
        for sc, (s, k, b, col, plo, phi) in enumerate(scol_rows):
            piece_id[(b, k, col)] = sc
        ecol = gslot // CH
        epart = gslot % CH
        esc = np.array([piece_id[(b, k, col)]
                        for b, k, col in zip(bs_, ks_, ecol)])
        rlT[epart, esc] = ds_
        per_core.append({
            "rowlocT": rlT.astype(ml_dtypes.bfloat16),
            "xT_self": np.ascontiguousarray(xT_pre[:, perm[cid * npc:(cid + 1) * npc]]),
            "dis_blk": _dis_blocks(dis[perm[cid * npc:(cid + 1) * npc]], nblk),
            **idx_arrs,
        })

    shared = {
        "x_pre": x_pre,
        "w_bf": np.asarray(weight, np.float32).astype(ml_dtypes.bfloat16),
        "iota_exp": np.repeat(np.arange(DBLK, dtype=np.float32), K_TT)[None]
        .repeat(CH, 0).astype(ml_dtypes.bfloat16),
        "bias_b": np.tile(np.asarray(bias, np.float32)[None, :], (CH, 1)),
    }
    meta = dict(npc=npc, nblk=nblk, nk=nk, nsup=nsup, sup_cols=sup_cols,
                schedule=schedule, S_tot=S_tot, S_pad=S_pad, perm=perm,
                slots_bk=slots_bk)
    return per_core, shared, meta


def _dis_blocks(dis_local, nblk):
    out = np.zeros((CH, nblk), np.float32)
    for b in range(nblk):
        seg = dis_local[b * DBLK:(b + 1) * DBLK]
        out[:len(seg), b] = seg
    return out


def _build(n_nodes, meta):
    npc, nblk, nk, nsup = meta["npc"], meta["nblk"], meta["nk"], meta["nsup"]
    sup_cols, schedule = meta["sup_cols"], meta["schedule"]
    S_pad = meta["S_pad"]

    nc = bacc.Bacc(None, target_bir_lowering=False)
    dt = mybir.dt

    x_d = nc.dram_tensor("x_pre", [n_nodes, F], dt.bfloat16, kind="ExternalInput")
    w_d = nc.dram_tensor("w_bf", [F, F], dt.bfloat16, kind="ExternalInput")
    iota_d = nc.dram_tensor("iota_exp", [CH, DBLK * K_TT], dt.bfloat16,
                            kind="ExternalInput")
    bias_d = nc.dram_tensor("bias_b", [CH, F], dt.float32, kind="ExternalInput")
    rlT_d = nc.dram_tensor("rowlocT", [CH, S_pad], dt.bfloat16,
                           kind="ExternalInput")
    xts_d = nc.dram_tensor("xT_self", [F, npc], dt.bfloat16,
                           kind="ExternalInput")
    dis_d = nc.dram_tensor("dis_blk", [CH, nblk], dt.float32,
                           kind="ExternalInput")
    idx_d = {}
    for s in range(nsup):
        for k in range(nk):
            idx_d[(s, k)] = nc.dram_tensor(
                f"idx_{s}_{k}", [CH, int(sup_cols[s, k]) * 8], dt.int16,
                kind="ExternalInput")
    y_d = nc.dram_tensor("y", [npc, F], dt.float32, kind="ExternalOutput")

    x_src = [x_d[k * SRC_BLK: min((k + 1) * SRC_BLK, n_nodes), :]
             for k in range(nk)]

    with tile.TileContext(nc) as tc:
        with (
            tc.tile_pool(name="const", bufs=1) as constp,
            tc.tile_pool(name="slab", bufs=5) as slabp,
            tc.tile_pool(name="idxp", bufs=5) as idxp,
            tc.tile_pool(name="sp", bufs=6) as sp,
            tc.tile_pool(name="pre", bufs=4) as prep,
            tc.tile_pool(name="ob", bufs=4) as obp,
            tc.tile_pool(name="pagg", bufs=3, space="PSUM") as paggp,
            tc.tile_pool(name="pout", bufs=3, space="PSUM") as poutp,
        ):
            w_sb = constp.tile([F, F], dt.bfloat16, tag="w")
            nc.sync.dma_start(w_sb[:], w_d[:])
            iota_sb = constp.tile([CH, DBLK * K_TT], dt.bfloat16, tag="iota")
            nc.sync.dma_start(iota_sb[:], iota_d[:])
            bias_sb = constp.tile([CH, F], dt.float32, tag="bias")
            nc.sync.dma_start(bias_sb[:], bias_d[:])
            rlT_sb = constp.tile([CH, S_pad], dt.bfloat16, tag="rlT")
            nc.sync.dma_start(rlT_sb[:], rlT_d[:])
            xts_sb = constp.tile([F, npc], dt.bfloat16, tag="xts")
            nc.sync.dma_start(xts_sb[:], xts_d[:])
            dis_sb = constp.tile([CH, nblk], dt.float32, tag="dis")
            nc.sync.dma_start(dis_sb[:], dis_d[:])

            s_tiles = {}

            def ensure_s(sc):
                base = (sc // K_TT) * K_TT
                if base in s_tiles:
                    return s_tiles[base]
                st = sp.tile([CH, DBLK * K_TT], dt.bfloat16, tag="s",
                             name=f"s_{base}")
                o3 = st[:].rearrange("p (d k) -> p d k", k=K_TT)
                i3 = iota_sb[:].rearrange("p (d k) -> p d k", k=K_TT)
                r3 = (rlT_sb[:, base:base + K_TT].unsqueeze(1)
                      .broadcast_to([CH, DBLK, K_TT]))
                nc.vector.tensor_tensor(o3, i3, r3, mybir.AluOpType.is_equal)
                s_tiles[base] = st
                return st

            for s in range(nsup):
                blocks = list(range(s * SUPER, min((s + 1) * SUPER, nblk)))
                pa = paggp.tile([F, DBLK * len(blocks)], dt.float32,
                                tag="pagg", name=f"pagg_{s}")
                tot = {b: 0 for b in blocks}
                for k in range(nk):
                    for (_, _, b, _, _) in schedule[(s, k)]:
                        tot[b] += 1
                done = {b: 0 for b in blocks}
                for k in range(nk):
                    cols = int(sup_cols[s, k])
                    slab = slabp.tile([CH, cols, F], dt.bfloat16, tag="slab",
                                      name=f"slab_{s}_{k}")
                    it = idxp.tile([CH, cols * 8], dt.int16, tag="idx",
                                   name=f"it_{s}_{k}")
                    nc.sync.dma_start(it[:], idx_d[(s, k)][:])
                    nc.gpsimd.dma_gather(
                        slab[:], x_src[k], it[:], cols * CH, cols * CH, F,
                        single_packet=False)
                    for (col, scol, b, plo, phi) in schedule[(s, k)]:
                        st = ensure_s(scol)
                        s3 = st[:].rearrange("p (d k) -> p d k", k=K_TT)
                        h = (b - s * SUPER) * DBLK
                        nc.tensor.matmul(
                            pa[:, h: h + DBLK],
                            slab[:, col, :],
                            s3[:, :, scol % K_TT],
                            start=(done[b] == 0),
                            stop=(done[b] == tot[b] - 1),
                            skip_group_check=True,
                        )
                        done[b] += 1
                for b in blocks:
                    rows = min(DBLK, npc - b * DBLK)
                    h = (b - s * SUPER) * DBLK
                    pre = prep.tile([F, DBLK], dt.bfloat16, tag="pre",
                                    name=f"pre_{b}")
                    nc.vector.tensor_tensor(
                        pre[:, :rows], pa[:, h: h + rows],
                        xts_sb[:, b * DBLK: b * DBLK + rows],
                        mybir.AluOpType.add,
                    )
                    po = poutp.tile([CH, F], dt.float32, tag="pout",
                                    name=f"po_{b}")
                    nc.tensor.matmul(po[:rows, :], pre[:, :rows], w_sb[:],
                                     start=True, stop=True)
                    ob = obp.tile([CH, F], dt.float32, tag="ob",
                                  name=f"ob_{b}")
                    nc.scalar.activation(
                        ob[:rows, :], po[:rows, :],
                        mybir.ActivationFunctionType.Copy,
                        scale=dis_sb[:rows, b: b + 1])
                    ob2 = obp.tile([CH, F], dt.float32, tag="ob2",
                                   name=f"ob2_{b}")
                    nc.vector.tensor_tensor(
                        ob2[:rows, :], ob[:rows, :], bias_sb[:rows, :],
                        mybir.AluOpType.add)
                    nc.sync.dma_start(
                        y_d[b * DBLK: b * DBLK + rows, :], ob2[:rows, :])

    nc.compile()
    return nc


def kernel(x, edge_index, weight, bias, _return_nc=False):
    x = np.ascontiguousarray(np.asarray(x, dtype=np.float32))
    n_nodes = x.shape[0]
    per_core, shared, meta = _prep(x, edge_index, weight, bias, n_nodes)
    nc = _build(n_nodes, meta)
    in_maps = [{**shared, **per_core[cid]} for cid in range(N_CORES)]
    res = run_bass_kernel_spmd(nc, in_maps, core_ids=list(range(N_CORES)))
    ys = np.concatenate([res.results[cid]["y"] for cid in range(N_CORES)],
                        axis=0)
    out = np.empty_like(ys)
    out[meta["perm"]] = ys           # slot -> global dest
    if _return_nc:
        return out, nc, in_maps
    return out
